# revision 13
# baseline (speedup 1.0000x reference)
"""Masked multi-head attention on 8 Trainium2 NeuronCores.

Problem: B=2, H=12, S=2048, D=64 attention with an int32 {0,1} mask
broadcast over heads.  out = softmax(mask ? QK^T/8 : -inf) @ V.

Sharding (8 cores, no cross-core comm):
  core c -> (b = c>>2, head-group hg = (c>>1)&1 -> 6 heads, q-half qh = c&1
  -> 1024 queries).  Each core computes full attention (all 2048 keys) for
  its 6 heads x 1024 queries.

The ACT engine's exp stream is the hard floor; everything else is arranged
to keep it gapless and to minimize its instruction count:
  - Host prepacks all operands into the exact fp16 SBUF layouts (Q^T
    duplicated on both partition halves, K^T pair-stacked for the two PE
    row groups, V with a fused ones-column, mask as fp16 {0,1}).  DMA lands
    directly in resident SBUF tiles; no on-device casts or staging.
  - Scores live in a flat per-head column space (k-tile major): 32 matmul
    chunks of 512 columns -> 11 exp instructions of 1536 columns (vs 16 of
    1024: fewer fixed per-instruction overheads on the pacing engine) into
    one resident [128, 16K] probs tile per head (3 rotating).
  - probs *= mask {0,1} on VectorE at 2048-column grain (2x DVE mode);
    equivalent to -inf masking, and a fully-masked row cannot occur.
  - AV: lhsT = probsT [k,128q] stationary, rhs = [V | ones] [k,65]; col 64
    accumulates the denominator.  Two AV q-tile chains accumulate
    interleaved on alternating PSUM banks (hides the per-matmul
    accumulation turnaround: 145ns -> 55ns effective), split into k-halves
    emitted across exp slots of the next head so each slot's PE load stays
    under the exp pace.
  - Output is fp16 (host converts); exp table preloaded at t=0.
"""

import os
import sys

import numpy as np

for _p in ("/opt/trn_rl_repo",):
    if _p not in sys.path and os.path.isdir(_p):
        sys.path.insert(0, _p)

import concourse.bass as bass
import concourse.mybir as mybir
import concourse.tile as tile
from concourse import bacc
from concourse.bass_utils import run_bass_kernel_spmd

FP16 = mybir.dt.float16
F32 = mybir.dt.float32

B, H, S, D = 2, 12, 2048, 64
NCORES = 8
HPC = 6        # heads per core
QPC = 1024     # queries per core
KT = S // 128  # 16 k-tiles
PAIRS = KT // 2
QTILES = QPC // 128
FLAT = KT * QPC          # 16384 flat score columns per head
NCHUNK = FLAT // 512     # 32 matmul chunks
ETILES = 11              # 10x1536 + 1x1024 exp tiles
EW = 1536

_NC_CACHE = None


def build_bass():
    """Build the single-core Bass/Tile program (SPMD across 8 cores)."""
    nc = bacc.Bacc("TRN2", target_bir_lowering=False, debug=False)

    # Host-prepacked fp16 operands (layouts documented in _shard).
    qt = nc.declare_dram_parameter("qt", [128, HPC, QPC], FP16, isOutput=False)
    kt = nc.declare_dram_parameter("kt", [128, HPC, QPC], FP16, isOutput=False)
    vt_d = nc.declare_dram_parameter("vt", [128, HPC, KT * 65], FP16, isOutput=False)
    mt = nc.declare_dram_parameter("mt", [128, KT, QPC], FP16, isOutput=False)
    o = nc.declare_dram_parameter("o", [HPC, QPC, D], FP16, isOutput=True)

    with tile.TileContext(nc) as tc:
        with (
            tc.tile_pool(name="const", bufs=1) as const,
            tc.tile_pool(name="probs", bufs=3) as probs_pool,
            tc.tile_pool(name="outp", bufs=6) as outp,
            tc.tile_pool(name="psc", bufs=2, space="PSUM") as psc,
            tc.tile_pool(name="pav", bufs=2, space="PSUM") as pav,
        ):
            # Resident fp16 operands (DMA'd straight from HBM, no staging).
            qh = const.tile([128, HPC, QPC], FP16)
            kh = const.tile([128, HPC, QPC], FP16)
            vt = const.tile([128, HPC, KT, 65], FP16)
            mb = const.tile([128, KT, QPC], FP16)
            scratch = const.tile([1, 2], FP16)

            mbf = mb.rearrange("p k q -> p (k q)")

            # Preload the Exp table while the first DMAs are in flight.
            nc.gpsimd.memset(scratch[:, 0:1], 0.0)
            nc.scalar.activation(
                scratch[:, 1:2], scratch[:, 0:1],
                mybir.ActivationFunctionType.Exp,
            )

            def load_head(h):
                """Pure DMAs into the resident tiles.  Head 0 is column-split
                so the first QK chunk can start after ~10% of the bytes."""
                if h == 0:
                    nc.sync.dma_start(kh[:, 0, 0:128], kt[:, 0, 0:128])
                    nc.sync.dma_start(qh[:, 0, 0:512], qt[:, 0, 0:512])
                    nc.sync.dma_start(qh[:, 0, 512:QPC], qt[:, 0, 512:QPC])
                    nc.sync.dma_start(kh[:, 0, 128:QPC], kt[:, 0, 128:QPC])
                else:
                    nc.sync.dma_start(kh[:, h, :], kt[:, h, :])
                    nc.sync.dma_start(qh[:, h, :], qt[:, h, :])
                nc.sync.dma_start(
                    vt[:, h, :, :],
                    vt_d[:, h, :].rearrange("p (t c) -> p t c", c=65),
                )

            def load_mask(t0, nt):
                nc.sync.dma_start(
                    mb[:, t0 : t0 + nt, :], mt[:, t0 : t0 + nt, :]
                )

            av_state = {}

            def av_mm(h, prh, t0, avps, k0, k1):
                for k in range(k0, k1):
                    col = QPC * k + 128 * t0
                    for c in range(2):
                        nc.tensor.matmul(
                            avps[c][:],
                            prh[:, col + 128 * c : col + 128 * c + 128],
                            vt[:, h, k, :],
                            start=(k == 0),
                            stop=(k == KT - 1),
                        )

            def av_begin(h, prh, t0):
                """First k-half of two interleaved AV accumulation chains on
                alternating PSUM banks (hides accumulation turnaround); the
                k-split keeps each slot's PE load under the exp pace."""
                avp0 = pav.tile([128, 65], F32, tag="av")
                avp1 = pav.tile([128, 65], F32, tag="av")
                av_state[0] = (avp0, avp1)
                av_mm(h, prh, t0, av_state[0], 0, KT // 2)

            def av_end(h, prh, t0):
                avps = av_state[0]
                av_mm(h, prh, t0, avps, KT // 2, KT)
                for c in range(2):
                    rec = outp.tile([128, 1], F32, tag="rec")
                    nc.vector.reciprocal(rec[:], avps[c][:, 64:65])
                    osb = outp.tile([128, D], FP16, tag="os")
                    nc.vector.tensor_scalar_mul(osb[:], avps[c][:, 0:64], rec[:])
                    t = t0 + c
                    nc.sync.dma_start(
                        o[h, 128 * t : 128 * t + 128, :], osb[:]
                    )

            def qk_etile(h, e, prh):
                """One exp tile: up to 3 QK chunks of 512 flat columns into a
                [128,1536] PSUM tile, one exp into the flat probs tile."""
                width = min(EW, FLAT - EW * e)
                nck = width // 512
                sc = psc.tile([128, EW], F32, tag="sc")
                for i in range(nck):
                    c = 3 * e + i
                    ktile, qc = c // 2, c % 2
                    r, a = ktile % 2, ktile // 2
                    lo = 64 * r
                    nc.tensor.matmul(
                        sc[:, 512 * i : 512 * i + 512],
                        kh[lo : lo + 64, h, 128 * a : 128 * a + 128],
                        qh[lo : lo + 64, h, 512 * qc : 512 * qc + 512],
                        start=True,
                        stop=True,
                        tile_position=(lo, 0),
                    )
                nc.scalar.activation(
                    prh[:, EW * e : EW * e + width],
                    sc[:, 0:width],
                    mybir.ActivationFunctionType.Exp,
                    scale=0.125,
                )

            def mask_mul(h, j, prh):
                seg = prh[:, 2048 * j : 2048 * j + 2048]
                nc.vector.tensor_mul(
                    seg.rearrange("p (t q) -> p t q", t=2),
                    seg.rearrange("p (t q) -> p t q", t=2),
                    mb[:, 2 * j : 2 * j + 2, :],
                )

            load_head(0)
            load_head(1)
            for t0 in range(0, KT, 4):
                load_mask(t0, 4)

            prev = None  # (head, probs tile) of the previous head
            prs = {}

            def new_pr(h):
                prh = probs_pool.tile([128, FLAT], FP16, tag="pp")
                prs[h] = prh
                return prh

            qk_etile(0, 0, new_pr(0))
            for h in range(HPC):
                prh = prs[h]
                ttj = 0   # next mask segment to emit
                avn = 0   # next AV half-step to emit
                for e in range(1, ETILES):
                    if h + 2 <= HPC - 1 and e == 5:
                        load_head(h + 2)
                    qk_etile(h, e, prh)
                    if e == ETILES - 2 and h + 1 < HPC:
                        # Next head's first tile: its chunks wait on a psc
                        # buffer freed by exp(h, e-1); the wait hides under
                        # the short e=10 tile emitted (and exp'd) after it.
                        qk_etile(h + 1, 0, new_pr(h + 1))
                    while (ttj + 1) * 2048 <= (e + 1) * EW and ttj < PAIRS:
                        mask_mul(h, ttj, prh)
                        ttj += 1
                    if prev is not None:
                        while avn < 8 and ((e + 1) * 8) // ETILES > avn:
                            t0 = (avn // 2) * 2
                            if avn % 2 == 0:
                                av_begin(prev[0], prev[1], t0)
                            else:
                                av_end(prev[0], prev[1], t0)
                            avn += 1
                prev = (h, prh)
            for t0 in range(0, QTILES, 2):
                av_begin(prev[0], prev[1], t0)
                av_end(prev[0], prev[1], t0)

    nc.compile()
    return nc


def _shard(c, Q, K, V, mask):
    """Host-side prepack into the exact fp16 SBUF layouts."""
    b, hg, qhalf = c >> 2, (c >> 1) & 1, c & 1
    hs = slice(hg * HPC, hg * HPC + HPC)
    qs = slice(qhalf * QPC, qhalf * QPC + QPC)

    # qh[64r+d, h, q] = Q[b, h, q, d]  (duplicated on both partition halves)
    qtp = np.ascontiguousarray(
        Q[b, hs, qs, :].transpose(2, 0, 1).astype(np.float16)
    )  # [64, HPC, QPC]
    qfull = np.concatenate([qtp, qtp], axis=0)  # [128, HPC, QPC]

    # kh[64r+d, h, 128a+c] = K[b, h, (2a+r)*128+c, d]  (pair-stacked)
    Ks = K[b, hs, :, :].astype(np.float16)  # [HPC, S, D]
    khp = (
        Ks.reshape(HPC, PAIRS, 2, 128, D)
        .transpose(2, 4, 0, 1, 3)  # [r, d, h, a, c]
        .reshape(128, HPC, QPC)
    )
    khp = np.ascontiguousarray(khp)

    # vt[p, h, t, 0:64] = V[b, h, t*128+p, :], vt[..., 64] = 1
    vtp = np.ones((128, HPC, KT, 65), dtype=np.float16)
    vtp[:, :, :, :64] = (
        V[b, hs, :, :].astype(np.float16).reshape(HPC, KT, 128, D).transpose(2, 0, 1, 3)
    )
    vtp = np.ascontiguousarray(vtp.reshape(128, HPC, KT * 65))

    # mb[p, t, q] = mask[b, 0, q0+q, t*128+p]  as fp16 {0,1}
    mtp = np.ascontiguousarray(
        mask[b, 0, qs, :].T.reshape(KT, 128, QPC).transpose(1, 0, 2).astype(np.float16)
    )

    return {"qt": qfull, "kt": khp, "vt": vtp, "mt": mtp}


def get_nc():
    global _NC_CACHE
    if _NC_CACHE is None:
        _NC_CACHE = build_bass()
    return _NC_CACHE


def kernel(Q, K, V, mask):
    Q = np.asarray(Q, dtype=np.float32)
    K = np.asarray(K, dtype=np.float32)
    V = np.asarray(V, dtype=np.float32)
    mask = np.asarray(mask, dtype=np.int32)

    in_maps = [_shard(c, Q, K, V, mask) for c in range(NCORES)]
    res = run_bass_kernel_spmd(get_nc(), in_maps, list(range(NCORES))).results

    out = np.empty((B, H, S, D), dtype=np.float32)
    for c in range(NCORES):
        b, hg, qhalf = c >> 2, (c >> 1) & 1, c & 1
        out[b, hg * HPC : hg * HPC + HPC, qhalf * QPC : qhalf * QPC + QPC, :] = (
            res[c]["o"].astype(np.float32)
        )
    return out


# revision 14
# speedup vs baseline: 1.0738x; 1.0738x over previous
"""Masked multi-head attention on 8 Trainium2 NeuronCores.

Problem: B=2, H=12, S=2048, D=64 attention with an int32 {0,1} mask
broadcast over heads.  out = softmax(mask ? QK^T/8 : -inf) @ V.

Sharding (8 cores, no cross-core comm):
  core c -> (b = c>>2, head-group hg = (c>>1)&1 -> 6 heads, q-half qh = c&1
  -> 1024 queries).  Each core computes full attention (all 2048 keys) for
  its 6 heads x 1024 queries.

The ACT engine's exp stream is the hard floor; everything else is arranged
to keep it gapless and to minimize its instruction count:
  - Host prepacks all operands into the exact fp16 SBUF layouts (Q^T
    duplicated on both partition halves, K^T pair-stacked for the two PE
    row groups, V with a fused ones-column, mask as fp16 {0,1}).  DMA lands
    directly in resident SBUF tiles; no on-device casts or staging.
  - Scores live in a flat per-head column space (k-tile major): 32 matmul
    chunks of 512 columns -> 11 exp instructions of 1536 columns (vs 16 of
    1024: fewer fixed per-instruction overheads on the pacing engine) into
    one resident [128, 16K] probs tile per head (3 rotating).
  - probs *= mask {0,1} on VectorE at 2048-column grain (2x DVE mode);
    equivalent to -inf masking, and a fully-masked row cannot occur.
  - AV: lhsT = probsT [k,128q] stationary, rhs = [V | ones] [k,65]; col 64
    accumulates the denominator.  Two AV q-tile chains accumulate
    interleaved on alternating PSUM banks (hides the per-matmul
    accumulation turnaround: 145ns -> 55ns effective), split into k-halves
    emitted across exp slots of the next head so each slot's PE load stays
    under the exp pace.
  - Output is fp16 (host converts); exp table preloaded at t=0.
"""

import os
import sys

import numpy as np

for _p in ("/opt/trn_rl_repo",):
    if _p not in sys.path and os.path.isdir(_p):
        sys.path.insert(0, _p)

import concourse.bass as bass
import concourse.mybir as mybir
import concourse.tile as tile
from concourse import bacc
from concourse.bass_utils import run_bass_kernel_spmd

FP16 = mybir.dt.float16
F32 = mybir.dt.float32

B, H, S, D = 2, 12, 2048, 64
NCORES = 8
HPC = 6        # heads per core
QPC = 1024     # queries per core
KT = S // 128  # 16 k-tiles
PAIRS = KT // 2
QTILES = QPC // 128
FLAT = KT * QPC          # 16384 flat score columns per head
NCHUNK = FLAT // 512     # 32 matmul chunks
ETILES = 11              # 10x1536 + 1x1024 exp tiles
EW = 1536

_NC_CACHE = None


def build_bass():
    """Build the single-core Bass/Tile program (SPMD across 8 cores)."""
    nc = bacc.Bacc("TRN2", target_bir_lowering=False, debug=False)

    # Host-prepacked fp16 operands (layouts documented in _shard).
    qt = nc.declare_dram_parameter("qt", [128, HPC, QPC], FP16, isOutput=False)
    kt = nc.declare_dram_parameter("kt", [128, HPC, QPC], FP16, isOutput=False)
    vt_d = nc.declare_dram_parameter("vt", [128, HPC, KT * 65], FP16, isOutput=False)
    mt = nc.declare_dram_parameter("mt", [128, KT, QPC], FP16, isOutput=False)
    o = nc.declare_dram_parameter("o", [HPC, QPC, D], FP16, isOutput=True)

    with tile.TileContext(nc) as tc:
        with (
            tc.tile_pool(name="const", bufs=1) as const,
            tc.tile_pool(name="probs", bufs=3) as probs_pool,
            tc.tile_pool(name="outp", bufs=6) as outp,
            tc.tile_pool(name="psc", bufs=2, space="PSUM") as psc,
            tc.tile_pool(name="pav", bufs=2, space="PSUM") as pav,
        ):
            # Resident fp16 operands (DMA'd straight from HBM, no staging).
            qh = const.tile([128, HPC, QPC], FP16)
            kh = const.tile([128, HPC, QPC], FP16)
            vt = const.tile([128, HPC, KT, 65], FP16)
            mb = const.tile([128, KT, QPC], FP16)
            scratch = const.tile([1, 2], FP16)

            mbf = mb.rearrange("p k q -> p (k q)")

            # Preload the Exp table while the first DMAs are in flight.
            nc.gpsimd.memset(scratch[:, 0:1], 0.0)
            nc.scalar.activation(
                scratch[:, 1:2], scratch[:, 0:1],
                mybir.ActivationFunctionType.Exp,
            )

            def load_head(h):
                """Pure DMAs into the resident tiles.  Head 0 is column-split
                so the first QK chunk can start after ~10% of the bytes."""
                if h == 0:
                    # e=0 needs kh cols 0:256 (k-tiles 0-2) + qh 0:512 only.
                    nc.sync.dma_start(kh[:, 0, 0:256], kt[:, 0, 0:256])
                    nc.sync.dma_start(qh[:, 0, 0:512], qt[:, 0, 0:512])
                    nc.sync.dma_start(kh[:, 0, 256:QPC], kt[:, 0, 256:QPC])
                    nc.sync.dma_start(qh[:, 0, 512:QPC], qt[:, 0, 512:QPC])
                else:
                    nc.sync.dma_start(kh[:, h, :], kt[:, h, :])
                    nc.sync.dma_start(qh[:, h, :], qt[:, h, :])
                nc.sync.dma_start(
                    vt[:, h, :, :],
                    vt_d[:, h, :].rearrange("p (t c) -> p t c", c=65),
                )

            def load_mask(t0, nt):
                nc.sync.dma_start(
                    mb[:, t0 : t0 + nt, :], mt[:, t0 : t0 + nt, :]
                )

            av_state = {}

            def av_mm(h, prh, t0, avps, k0, k1):
                qc = t0 // 4
                for k in range(k0, k1):
                    col = 8192 * qc + 512 * k + 128 * (t0 % 4)
                    for c in range(2):
                        nc.tensor.matmul(
                            avps[c][:],
                            prh[:, col + 128 * c : col + 128 * c + 128],
                            vt[:, h, k, :],
                            start=(k == 0),
                            stop=(k == KT - 1),
                        )

            def av_begin(h, prh, t0):
                """First k-half of two interleaved AV accumulation chains on
                alternating PSUM banks (hides accumulation turnaround); the
                k-split keeps each slot's PE load under the exp pace."""
                avp0 = pav.tile([128, 65], F32, tag="av")
                avp1 = pav.tile([128, 65], F32, tag="av")
                av_state[0] = (avp0, avp1)
                av_mm(h, prh, t0, av_state[0], 0, KT // 2)

            def av_end(h, prh, t0):
                avps = av_state[0]
                av_mm(h, prh, t0, avps, KT // 2, KT)
                for c in range(2):
                    rec = outp.tile([128, 1], F32, tag="rec")
                    nc.vector.reciprocal(rec[:], avps[c][:, 64:65])
                    osb = outp.tile([128, D], FP16, tag="os")
                    nc.vector.tensor_scalar_mul(osb[:], avps[c][:, 0:64], rec[:])
                    t = t0 + c
                    nc.sync.dma_start(
                        o[h, 128 * t : 128 * t + 128, :], osb[:]
                    )

            def qk_etile(h, e, prh):
                """One exp tile: up to 3 QK chunks of 512 flat columns into a
                [128,1536] PSUM tile, one exp into the flat probs tile."""
                width = min(EW, FLAT - EW * e)
                nck = width // 512
                sc = psc.tile([128, EW], F32, tag="sc")
                for i in range(nck):
                    c = 3 * e + i
                    qc, ktile = c // 16, c % 16
                    r, a = ktile % 2, ktile // 2
                    lo = 64 * r
                    nc.tensor.matmul(
                        sc[:, 512 * i : 512 * i + 512],
                        kh[lo : lo + 64, h, 128 * a : 128 * a + 128],
                        qh[lo : lo + 64, h, 512 * qc : 512 * qc + 512],
                        start=True,
                        stop=True,
                        tile_position=(lo, 0),
                    )
                nc.scalar.activation(
                    prh[:, EW * e : EW * e + width],
                    sc[:, 0:width],
                    mybir.ActivationFunctionType.Exp,
                    scale=0.125,
                )

            def mask_mul(h, j, prh):
                qc, kt0 = j // 4, 4 * (j % 4)
                seg = prh[:, 2048 * j : 2048 * j + 2048]
                nc.vector.tensor_mul(
                    seg.rearrange("p (t q) -> p t q", t=4),
                    seg.rearrange("p (t q) -> p t q", t=4),
                    mb[:, kt0 : kt0 + 4, 512 * qc : 512 * qc + 512],
                )

            load_head(0)
            load_head(1)
            for t0 in range(0, KT, 4):
                load_mask(t0, 4)

            prev = None  # (head, probs tile) of the previous head
            prs = {}

            def new_pr(h):
                prh = probs_pool.tile([128, FLAT], FP16, tag="pp")
                prs[h] = prh
                return prh

            qk_etile(0, 0, new_pr(0))
            for h in range(HPC):
                prh = prs[h]
                ttj = 0   # next mask segment to emit
                avn = 0   # next AV half-step to emit
                for e in range(1, ETILES):
                    if h + 2 <= HPC - 1 and e == 5:
                        load_head(h + 2)
                    qk_etile(h, e, prh)
                    if e == ETILES - 2 and h + 1 < HPC:
                        # Next head's first tile: its chunks wait on a psc
                        # buffer freed by exp(h, e-1); the wait hides under
                        # the short e=10 tile emitted (and exp'd) after it.
                        qk_etile(h + 1, 0, new_pr(h + 1))
                    while (ttj + 1) * 2048 <= (e + 1) * EW and ttj < PAIRS:
                        mask_mul(h, ttj, prh)
                        ttj += 1
                    if prev is not None:
                        while avn < 8 and ((e + 1) * 8) // ETILES > avn:
                            t0 = (avn // 2) * 2
                            if avn % 2 == 0:
                                av_begin(prev[0], prev[1], t0)
                            else:
                                av_end(prev[0], prev[1], t0)
                            avn += 1
                prev = (h, prh)
            for t0 in range(0, QTILES, 2):
                av_begin(prev[0], prev[1], t0)
                av_end(prev[0], prev[1], t0)

    nc.compile()
    return nc


def _shard(c, Q, K, V, mask):
    """Host-side prepack into the exact fp16 SBUF layouts."""
    b, hg, qhalf = c >> 2, (c >> 1) & 1, c & 1
    hs = slice(hg * HPC, hg * HPC + HPC)
    qs = slice(qhalf * QPC, qhalf * QPC + QPC)

    # qh[64r+d, h, q] = Q[b, h, q, d]  (duplicated on both partition halves)
    qtp = np.ascontiguousarray(
        Q[b, hs, qs, :].transpose(2, 0, 1).astype(np.float16)
    )  # [64, HPC, QPC]
    qfull = np.concatenate([qtp, qtp], axis=0)  # [128, HPC, QPC]

    # kh[64r+d, h, 128a+c] = K[b, h, (2a+r)*128+c, d]  (pair-stacked)
    Ks = K[b, hs, :, :].astype(np.float16)  # [HPC, S, D]
    khp = (
        Ks.reshape(HPC, PAIRS, 2, 128, D)
        .transpose(2, 4, 0, 1, 3)  # [r, d, h, a, c]
        .reshape(128, HPC, QPC)
    )
    khp = np.ascontiguousarray(khp)

    # vt[p, h, t, 0:64] = V[b, h, t*128+p, :], vt[..., 64] = 1
    vtp = np.ones((128, HPC, KT, 65), dtype=np.float16)
    vtp[:, :, :, :64] = (
        V[b, hs, :, :].astype(np.float16).reshape(HPC, KT, 128, D).transpose(2, 0, 1, 3)
    )
    vtp = np.ascontiguousarray(vtp.reshape(128, HPC, KT * 65))

    # mb[p, t, q] = mask[b, 0, q0+q, t*128+p]  as fp16 {0,1}
    mtp = np.ascontiguousarray(
        mask[b, 0, qs, :].T.reshape(KT, 128, QPC).transpose(1, 0, 2).astype(np.float16)
    )

    return {"qt": qfull, "kt": khp, "vt": vtp, "mt": mtp}


def get_nc():
    global _NC_CACHE
    if _NC_CACHE is None:
        _NC_CACHE = build_bass()
    return _NC_CACHE


def kernel(Q, K, V, mask):
    Q = np.asarray(Q, dtype=np.float32)
    K = np.asarray(K, dtype=np.float32)
    V = np.asarray(V, dtype=np.float32)
    mask = np.asarray(mask, dtype=np.int32)

    in_maps = [_shard(c, Q, K, V, mask) for c in range(NCORES)]
    res = run_bass_kernel_spmd(get_nc(), in_maps, list(range(NCORES))).results

    out = np.empty((B, H, S, D), dtype=np.float32)
    for c in range(NCORES):
        b, hg, qhalf = c >> 2, (c >> 1) & 1, c & 1
        out[b, hg * HPC : hg * HPC + HPC, qhalf * QPC : qhalf * QPC + QPC, :] = (
            res[c]["o"].astype(np.float32)
        )
    return out


# revision 15
# speedup vs baseline: 1.2453x; 1.1597x over previous
"""Masked multi-head attention on 8 Trainium2 NeuronCores.

Problem: B=2, H=12, S=2048, D=64 attention with an int32 {0,1} mask
broadcast over heads.  out = softmax(mask ? QK^T/8 : -inf) @ V.

Sharding (8 cores, no cross-core comm):
  core c -> (b = c>>2, head-group hg = (c>>1)&1 -> 6 heads, q-half qh = c&1
  -> 1024 queries).  Each core computes full attention (all 2048 keys) for
  its 6 heads x 1024 queries.

The ACT engine's exp stream is the hard floor; everything else is arranged
to keep it gapless and to minimize its instruction count:
  - Host prepacks all operands into the exact fp16 SBUF layouts (Q^T
    duplicated on both partition halves, K^T pair-stacked for the two PE
    row groups, V with a fused ones-column, mask as fp16 {0,1}).  DMA lands
    directly in resident SBUF tiles; no on-device casts or staging.
  - Scores live in a flat per-head column space (q-chunk major, so the
    first exp tile needs only 1/4 of head 0's K/Q bytes): 32 matmul chunks
    of 512 columns -> 11 exp instructions of 1536 columns (vs 16 of 1024:
    fewer fixed per-instruction overheads on the pacing engine) into one
    resident [128, 16K] probs tile per head (3 rotating; whole-head WAR
    deps instead of per-pair cut the ACT queue's semaphore time ~4x).
    The next head's first exp tile is emitted before the current head's
    short last tile, so its PSUM WAR latency hides at head boundaries.
  - probs *= mask {0,1} on VectorE at 2048-column grain (2x DVE mode);
    equivalent to -inf masking, and a fully-masked row cannot occur.
  - AV: lhsT = probsT [k,128q] stationary, rhs = [V | ones] [k,65]; col 64
    accumulates the denominator.  Two AV q-tile chains accumulate
    interleaved on alternating PSUM banks (hides the per-matmul
    accumulation turnaround: 145ns -> 55ns effective), split into k-halves
    emitted across exp slots of the next head so each slot's PE load stays
    under the exp pace.
  - Output is fp16 (host converts); exp table preloaded at t=0.
"""

import os
import sys

import numpy as np

for _p in ("/opt/trn_rl_repo",):
    if _p not in sys.path and os.path.isdir(_p):
        sys.path.insert(0, _p)

import concourse.bass as bass
import concourse.mybir as mybir
import concourse.tile as tile
from concourse import bacc
from concourse.bass_utils import run_bass_kernel_spmd

FP16 = mybir.dt.float16
F32 = mybir.dt.float32

B, H, S, D = 2, 12, 2048, 64
NCORES = 8
HPC = 6        # heads per core
QPC = 1024     # queries per core
KT = S // 128  # 16 k-tiles
PAIRS = KT // 2
QTILES = QPC // 128
FLAT = KT * QPC          # 16384 flat score columns per head
NCHUNK = FLAT // 512     # 32 matmul chunks
ETILES = 11              # 10x1536 + 1x1024 exp tiles
EW = 1536

_NC_CACHE = None


def build_bass():
    """Build the single-core Bass/Tile program (SPMD across 8 cores)."""
    nc = bacc.Bacc("TRN2", target_bir_lowering=False, debug=False)

    # Host-prepacked fp16 operands (layouts documented in _shard).
    qt = nc.declare_dram_parameter("qt", [128, HPC, QPC], FP16, isOutput=False)
    kt = nc.declare_dram_parameter("kt", [128, HPC, QPC], FP16, isOutput=False)
    vt_d = nc.declare_dram_parameter("vt", [128, HPC, KT * 65], FP16, isOutput=False)
    mt = nc.declare_dram_parameter("mt", [128, KT, QPC], FP16, isOutput=False)
    o = nc.declare_dram_parameter("o", [HPC, QPC, D], FP16, isOutput=True)

    with tile.TileContext(nc) as tc:
        with (
            tc.tile_pool(name="const", bufs=1) as const,
            tc.tile_pool(name="probs", bufs=3) as probs_pool,
            tc.tile_pool(name="outp", bufs=6) as outp,
            tc.tile_pool(name="psc", bufs=2, space="PSUM") as psc,
            tc.tile_pool(name="pav", bufs=2, space="PSUM") as pav,
        ):
            # Resident fp16 operands (DMA'd straight from HBM, no staging).
            qh = const.tile([128, HPC, QPC], FP16)
            kh = const.tile([128, HPC, QPC], FP16)
            vt = const.tile([128, HPC, KT, 65], FP16)
            mb = const.tile([128, KT, QPC], FP16)
            scratch = const.tile([1, 2], FP16)

            mbf = mb.rearrange("p k q -> p (k q)")

            # Preload the Exp table while the first DMAs are in flight.
            nc.gpsimd.memset(scratch[:, 0:1], 0.0)
            nc.scalar.activation(
                scratch[:, 1:2], scratch[:, 0:1],
                mybir.ActivationFunctionType.Exp,
            )

            def load_head(h):
                """Pure DMAs into the resident tiles.  Head 0 is column-split
                so the first QK chunk can start after ~10% of the bytes."""
                if h == 0:
                    # e=0 needs kh cols 0:256 (k-tiles 0-2) + qh 0:512 only.
                    nc.sync.dma_start(kh[:, 0, 0:256], kt[:, 0, 0:256])
                    nc.sync.dma_start(qh[:, 0, 0:512], qt[:, 0, 0:512])
                    nc.sync.dma_start(kh[:, 0, 256:QPC], kt[:, 0, 256:QPC])
                    nc.sync.dma_start(qh[:, 0, 512:QPC], qt[:, 0, 512:QPC])
                else:
                    nc.sync.dma_start(kh[:, h, :], kt[:, h, :])
                    nc.sync.dma_start(qh[:, h, :], qt[:, h, :])
                nc.sync.dma_start(
                    vt[:, h, :, :],
                    vt_d[:, h, :].rearrange("p (t c) -> p t c", c=65),
                )

            def load_mask(t0, nt):
                nc.sync.dma_start(
                    mb[:, t0 : t0 + nt, :], mt[:, t0 : t0 + nt, :]
                )

            av_state = {}

            def av_mm(h, prh, t0, avps, k0, k1):
                qc = t0 // 4
                for k in range(k0, k1):
                    col = 8192 * qc + 512 * k + 128 * (t0 % 4)
                    for c in range(2):
                        nc.tensor.matmul(
                            avps[c][:],
                            prh[:, col + 128 * c : col + 128 * c + 128],
                            vt[:, h, k, :],
                            start=(k == 0),
                            stop=(k == KT - 1),
                        )

            def av_begin(h, prh, t0):
                """First k-half of two interleaved AV accumulation chains on
                alternating PSUM banks (hides accumulation turnaround); the
                k-split keeps each slot's PE load under the exp pace."""
                avp0 = pav.tile([128, 65], F32, tag="av")
                avp1 = pav.tile([128, 65], F32, tag="av")
                av_state[0] = (avp0, avp1)
                av_mm(h, prh, t0, av_state[0], 0, KT // 2)

            def av_end(h, prh, t0):
                avps = av_state[0]
                av_mm(h, prh, t0, avps, KT // 2, KT)
                for c in range(2):
                    rec = outp.tile([128, 1], F32, tag="rec")
                    nc.vector.reciprocal(rec[:], avps[c][:, 64:65])
                    osb = outp.tile([128, D], FP16, tag="os")
                    nc.vector.tensor_scalar_mul(osb[:], avps[c][:, 0:64], rec[:])
                    t = t0 + c
                    nc.sync.dma_start(
                        o[h, 128 * t : 128 * t + 128, :], osb[:]
                    )

            def qk_etile(h, e, prh):
                """One exp tile: up to 3 QK chunks of 512 flat columns into a
                [128,1536] PSUM tile, one exp into the flat probs tile."""
                width = min(EW, FLAT - EW * e)
                nck = width // 512
                sc = psc.tile([128, EW], F32, tag="sc")
                for i in range(nck):
                    c = 3 * e + i
                    qc, ktile = c // 16, c % 16
                    r, a = ktile % 2, ktile // 2
                    lo = 64 * r
                    nc.tensor.matmul(
                        sc[:, 512 * i : 512 * i + 512],
                        kh[lo : lo + 64, h, 128 * a : 128 * a + 128],
                        qh[lo : lo + 64, h, 512 * qc : 512 * qc + 512],
                        start=True,
                        stop=True,
                        tile_position=(lo, 0),
                    )
                nc.scalar.activation(
                    prh[:, EW * e : EW * e + width],
                    sc[:, 0:width],
                    mybir.ActivationFunctionType.Exp,
                    scale=0.125,
                )

            def mask_mul(h, j, prh):
                qc, kt0 = j // 4, 4 * (j % 4)
                seg = prh[:, 2048 * j : 2048 * j + 2048]
                nc.vector.tensor_mul(
                    seg.rearrange("p (t q) -> p t q", t=4),
                    seg.rearrange("p (t q) -> p t q", t=4),
                    mb[:, kt0 : kt0 + 4, 512 * qc : 512 * qc + 512],
                )

            load_head(0)
            load_head(1)
            for t0 in range(0, KT, 4):
                load_mask(t0, 4)

            prev = None  # (head, probs tile) of the previous head
            prs = {}

            def new_pr(h):
                prh = probs_pool.tile([128, FLAT], FP16, tag="pp")
                prs[h] = prh
                return prh

            qk_etile(0, 0, new_pr(0))
            for h in range(HPC):
                prh = prs[h]
                ttj = 0   # next mask segment to emit
                avn = 0   # next AV half-step to emit
                for e in range(1, ETILES):
                    if h + 2 <= HPC - 1 and e == 5:
                        load_head(h + 2)
                    qk_etile(h, e, prh)
                    if e == ETILES - 2 and h + 1 < HPC:
                        # Next head's first tile: its chunks wait on a psc
                        # buffer freed by exp(h, e-1); the wait hides under
                        # the short e=10 tile emitted (and exp'd) after it.
                        qk_etile(h + 1, 0, new_pr(h + 1))
                    while (ttj + 1) * 2048 <= (e + 1) * EW and ttj < PAIRS:
                        mask_mul(h, ttj, prh)
                        ttj += 1
                    if prev is not None:
                        while avn < 8 and ((e + 1) * 8) // ETILES > avn:
                            t0 = (avn // 2) * 2
                            if avn % 2 == 0:
                                av_begin(prev[0], prev[1], t0)
                            else:
                                av_end(prev[0], prev[1], t0)
                            avn += 1
                prev = (h, prh)
            for t0 in range(0, QTILES, 2):
                av_begin(prev[0], prev[1], t0)
                av_end(prev[0], prev[1], t0)

    nc.compile()
    return nc


def _shard(c, Q, K, V, mask):
    """Host-side prepack into the exact fp16 SBUF layouts."""
    b, hg, qhalf = c >> 2, (c >> 1) & 1, c & 1
    hs = slice(hg * HPC, hg * HPC + HPC)
    qs = slice(qhalf * QPC, qhalf * QPC + QPC)

    # qh[64r+d, h, q] = Q[b, h, q, d]  (duplicated on both partition halves)
    qtp = np.ascontiguousarray(
        Q[b, hs, qs, :].transpose(2, 0, 1).astype(np.float16)
    )  # [64, HPC, QPC]
    qfull = np.concatenate([qtp, qtp], axis=0)  # [128, HPC, QPC]

    # kh[64r+d, h, 128a+c] = K[b, h, (2a+r)*128+c, d]  (pair-stacked)
    Ks = K[b, hs, :, :].astype(np.float16)  # [HPC, S, D]
    khp = (
        Ks.reshape(HPC, PAIRS, 2, 128, D)
        .transpose(2, 4, 0, 1, 3)  # [r, d, h, a, c]
        .reshape(128, HPC, QPC)
    )
    khp = np.ascontiguousarray(khp)

    # vt[p, h, t, 0:64] = V[b, h, t*128+p, :], vt[..., 64] = 1
    vtp = np.ones((128, HPC, KT, 65), dtype=np.float16)
    vtp[:, :, :, :64] = (
        V[b, hs, :, :].astype(np.float16).reshape(HPC, KT, 128, D).transpose(2, 0, 1, 3)
    )
    vtp = np.ascontiguousarray(vtp.reshape(128, HPC, KT * 65))

    # mb[p, t, q] = mask[b, 0, q0+q, t*128+p]  as fp16 {0,1}
    mtp = np.ascontiguousarray(
        mask[b, 0, qs, :].T.reshape(KT, 128, QPC).transpose(1, 0, 2).astype(np.float16)
    )

    return {"qt": qfull, "kt": khp, "vt": vtp, "mt": mtp}


def get_nc():
    global _NC_CACHE
    if _NC_CACHE is None:
        _NC_CACHE = build_bass()
    return _NC_CACHE


def kernel(Q, K, V, mask):
    Q = np.asarray(Q, dtype=np.float32)
    K = np.asarray(K, dtype=np.float32)
    V = np.asarray(V, dtype=np.float32)
    mask = np.asarray(mask, dtype=np.int32)

    in_maps = [_shard(c, Q, K, V, mask) for c in range(NCORES)]
    res = run_bass_kernel_spmd(get_nc(), in_maps, list(range(NCORES))).results

    out = np.empty((B, H, S, D), dtype=np.float32)
    for c in range(NCORES):
        b, hg, qhalf = c >> 2, (c >> 1) & 1, c & 1
        out[b, hg * HPC : hg * HPC + HPC, qhalf * QPC : qhalf * QPC + QPC, :] = (
            res[c]["o"].astype(np.float32)
        )
    return out
